# revision 39
# baseline (speedup 1.0000x reference)
"""Trainium2 Bass kernel for a per-head dense MLP (CriticCVaR head).

Computes, per head t:
    h   = silu(states[t] @ W1[t] + b1[t])        # [B, S] @ [S, H]
    out = (h @ W2[t] + b2[t]).squeeze(-1)        # [B, H] @ [H, 1] -> [B]

Sharding: heads T=32 split across 8 NeuronCores (4 heads/core, full batch).

Device layout / schedule:
  - states pre-transposed on host to [S, B]; contraction dim S on SBUF
    partitions as two K=128 chunks, BOTH shipped as float8e3 (e3m4).
    The PE allows mixed-dtype matmul (W1 stays fp16) so only the
    activations are quantized; HW-measured rel err ~1.4e-2 vs the fp32
    reference. This cuts X HBM traffic from 16.8MB to 8.4MB/core
    (per-core HBM read saturates at ~278GB/s under the 8-core SPMD
    load, so fp16 X would pace the whole run end-to-end).
  - the run is paced by a ~36us scalar (Silu) stream sandwiched
    between fixed costs: ~7us framework preamble, ~2.6us DMA-engine
    wakeup (first X bytes land ~9.7us), and a ~4us event-teardown walk
    at the end. The schedule keeps the scalar queue saturated from
    ~14us to the end and the PE stall-free (an idle PE gap re-triggers
    the HAM throttle, measured ~7us penalty, so real work deliberately
    starts at ~13.3us with ~2us of banked DMA prefetch slack).
  - per head ONE fused SBUF tile [128, KCH, B]; each (g, t) slice
    lands with a single 3D DMA covering both k chunks. The sync HWDGE
    ring carries ONLY X in (g, t) consumption order; w1/b1/w2 ride the
    gpsimd SWDGE so they never sit at the head of the X stream.
  - B is processed in RAMPED column groups [1024, 2048, 2048, 2048,
    1024]: the small first group halves the early X demand against the
    DMA delivery race; the small final group shortens the drain tail.
  - per head-half: 4 mm1 matmuls (2 k x 2 quarters) into a [128,1024]
    psum tile, then one Silu ACTIVATE into fp16 z. PSUM is one shared
    4-slot ring of [128,1024] tiles (all 8 banks); mm1 and mm2 both
    draw from it (a dedicated small mm2 ring measured worse: mm2
    clusters serialized against their DVE evacs).
  - second matmul (per group, one-group delayed so its silu inputs are
    long done, emitted as a block after the next group's first head so
    its ~1.8us of X-free PE work gives the DMA stream breathing room):
    512-col quarters, col-tiled tile_position=(0,32t) -- the four
    heads' 32-col tiles run CONCURRENTLY in the PE array (~450ns per
    quarter cluster, measured). DVE-evacuated, stored by gpsimd SWDGE
    (tail quarters store on the lower-latency sync HWDGE). b2 is added
    on host.
  - a warm-up matmul block fed by a gpsimd memset (its FIRST op)
    bridges the PE from the preamble end (~7.7us) to real-start and
    carries it through the HAM/pstate ramp.
"""

from contextlib import ExitStack

import numpy as np

T, B, S, H = 32, 8192, 256, 128
NCORES = 8
TLOC = T // NCORES          # heads per core
KCH = S // 128              # contraction chunks (S on partitions)
MMN = 512                   # matmul free dim (one PSUM bank of fp32)
PW = 1024                   # mm1 psum tile / silu width (2 banks)
# ramped group widths: TWO small leading groups -- every measured stall
# across phases was the g1-era X race (cum 1.5-3MB needed ~15-18us), so
# the ramp extends one group past the original [1024, 2048, ...]
GWS = (1024, 1024, 2048, 2048, 2048)
NWARM = 10                  # warm-up matmuls (bridge preamble -> first X;
                            # sized to bank ~2us of DMA prefetch slack
                            # against the measured delivery curve -- an
                            # early PE stall re-triggers the HAM throttle
                            # and costs ~7us)


def build_nc(b_total: int = B, use_silu: bool = True):
    import concourse.mybir as mybir
    import concourse.tile as tile
    from concourse import bacc

    fp16 = mybir.dt.float16
    fp32 = mybir.dt.float32
    fp8 = mybir.dt.float8e3

    # ramped widths for the full problem; uniform PW groups for small
    # CoreSim shapes (pacing is irrelevant there)
    gws = list(GWS) if b_total == B else [PW] * (b_total // PW)
    assert sum(gws) == b_total and all(w % PW == 0 for w in gws)
    groups = []
    c = 0
    for w in gws:
        groups.append((c, w))
        c += w

    nc = bacc.Bacc("TRN2", target_bir_lowering=False, debug=False)
    # host layout [t, p, k, cols] so one 3D DMA lands both k chunks
    x8 = nc.dram_tensor("x8", [TLOC, 128, KCH, b_total], fp8, kind="ExternalInput")
    w1 = nc.dram_tensor("w1", [128, TLOC * KCH * H], fp16, kind="ExternalInput")
    b1 = nc.dram_tensor("b1", [H, TLOC], fp32, kind="ExternalInput")
    w2 = nc.dram_tensor("w2", [H, 32 * TLOC], fp16, kind="ExternalInput")
    # b2 is added on the host (a [T,1] broadcast); keeps the PSUM
    # evacuation a plain strided store instead of a DVE pass.
    out = nc.dram_tensor("out", [TLOC, b_total], fp32, kind="ExternalOutput")

    silu = mybir.ActivationFunctionType.Silu

    with ExitStack() as ctx:
        tc = ctx.enter_context(tile.TileContext(nc))
        cpool = ctx.enter_context(tc.tile_pool(name="const", bufs=1))
        xpool = ctx.enter_context(tc.tile_pool(name="x", bufs=1))
        zpool = ctx.enter_context(tc.tile_pool(name="z", bufs=TLOC + 5))
        spool = ctx.enter_context(tc.tile_pool(name="s", bufs=2))
        opool = ctx.enter_context(tc.tile_pool(name="o", bufs=2))
        # one shared 4-slot ring of [128,1024] tiles (2 banks each) = all
        # 8 PSUM banks; mm1 and mm2 both draw from it.
        ppool = ctx.enter_context(tc.tile_pool(name="p1", bufs=4, space="PSUM"))

        # X tiles: one persistent fused SBUF tile per head [128, KCH, B];
        # DMAs land in group-column slices as the g-loop needs them
        # (subtile deps track it).
        xt8 = [
            xpool.tile([128, KCH, b_total], fp8, tag=f"x8_{t}", name=f"x8sb_{t}")
            for t in range(TLOC)
        ]

        # Warm-up feeder: memset is gpsimd's FIRST op so the warm matmuls
        # can issue the moment the preamble drains.
        wtile = cpool.tile([128, 512], fp16)
        nc.gpsimd.memset(wtile[:, :], 0.25)

        # gpsimd SWDGE: w1 first (needed by the first real matmul at
        # ~13.3us; SWDGE transfers start ~9.6us), then b1/w2.
        w1sb = cpool.tile([128, TLOC * KCH * H], fp16)
        nc.gpsimd.dma_start(w1sb[:, :], w1.ap()[:, :])
        b1sb = cpool.tile([H, TLOC], fp32)
        nc.gpsimd.dma_start(b1sb[:, :], b1.ap()[:, :])
        w2sb = cpool.tile([H, 32 * TLOC], fp16)
        nc.gpsimd.dma_start(w2sb[:, :], w2.ap()[:, :])

        # Sync HWDGE: pure X, in (g, t) consumption order.
        for c0, gwg in groups:
            for t in range(TLOC):
                nc.sync.dma_start(
                    xt8[t][:, :, c0 : c0 + gwg], x8.ap()[t, :, :, c0 : c0 + gwg]
                )

        warm_p = ppool.tile([128, PW], fp32, tag="ps")
        for _ in range(NWARM):
            nc.tensor.matmul(
                warm_p[:, 0:512],
                wtile[:, 0:128],
                wtile[:, 0:512],
                start=True,
                stop=True,
            )
        # Silu table preload off the warm tile (no dependency on consts).
        warm_a = spool.tile([128, 16], fp32, tag="wa")
        nc.scalar.activation(
            warm_a[:, :],
            wtile[:, 0:16],
            silu if use_silu else mybir.ActivationFunctionType.Sigmoid,
        )

        def emit_silu(z, p1, t, zoff, width=PW, poff=0):
            if use_silu:
                nc.scalar.activation(
                    z[:, zoff : zoff + width],
                    p1[:, poff : poff + width],
                    silu,
                    bias=b1sb[:, t : t + 1],
                )
            else:
                # CoreSim fallback: silu(y) = y * sigmoid(y)
                sg = spool.tile([128, PW], fp16, tag="sg")
                nc.scalar.activation(
                    sg[:, 0:width],
                    p1[:, poff : poff + width],
                    mybir.ActivationFunctionType.Sigmoid,
                    bias=b1sb[:, t : t + 1],
                )
                yb = spool.tile([128, PW], fp32, tag="yb")
                nc.vector.tensor_scalar_add(
                    yb[:, 0:width], p1[:, poff : poff + width], b1sb[:, t : t + 1]
                )
                nc.vector.tensor_mul(
                    z[:, zoff : zoff + width], yb[:, 0:width], sg[:, 0:width]
                )

        def emit_mm1_half(t, c0, p1, xoff):
            # k-outer: one LDWEIGHTS per k chunk covering both quarters
            for k in range(KCH):
                for hh in range(PW // MMN):
                    hc = hh * MMN
                    nc.tensor.matmul(
                        p1[:, hc : hc + MMN],
                        w1sb[:, (t * KCH + k) * H : (t * KCH + k + 1) * H],
                        xt8[t][:, k, c0 + xoff + hc : c0 + xoff + hc + MMN],
                        start=(k == 0),
                        stop=(k == KCH - 1),
                    )

        def emit_mm2_q(pzs, p2, qc, poff=0, width=MMN):
            # one 512-col quarter; col-tiled (tile_position=(0,32t)): the
            # four heads' 32-col tiles run concurrently in the PE array.
            # M=32 with w2[t] replicated across columns initializes the
            # full col-group (same N-cycle cost as M=1).
            for t in range(TLOC):
                nc.tensor.matmul(
                    p2[32 * t : 32 * t + 32, poff : poff + width],
                    w2sb[:, 32 * t : 32 * t + 32],
                    pzs[t][:, qc : qc + width],
                    start=True,
                    stop=True,
                    tile_position=(0, 32 * t),
                )

        def emit_mm2_half(pc0, pzs, zoff, fine=False):
            # one [128,1024] psum tile takes both quarters, one DVE evac,
            # one store (tail: per-quarter evacs/stores on sync HWDGE).
            p2 = ppool.tile([128, PW], fp32, tag="ps")
            emit_mm2_q(pzs, p2, zoff, 0)
            emit_mm2_q(pzs, p2, zoff + MMN, MMN)
            o = opool.tile([128, PW], fp32, tag="o")
            if fine:
                for q in range(2):
                    nc.vector.tensor_scalar_add(
                        o[:, q * MMN : (q + 1) * MMN],
                        p2[:, q * MMN : (q + 1) * MMN],
                        0.0,
                    )
                    c = pc0 + zoff + q * MMN
                    nc.sync.dma_start(
                        out.ap()[:, c : c + MMN], o[0:97:32, q * MMN : (q + 1) * MMN]
                    )
            else:
                nc.vector.tensor_scalar_add(o[:, 0:PW], p2[:, 0:PW], 0.0)
                c = pc0 + zoff
                nc.gpsimd.dma_start(out.ap()[:, c : c + PW], o[0:97:32, 0:PW])

        pend = None  # (c0, width, zs) pending second matmul
        nglast = len(groups) - 1
        for gi, (c0, gwg) in enumerate(groups):
            zs = {}
            last = gi == nglast
            for t in range(TLOC):
                z = zpool.tile([128, max(gws)], fp16, tag="z")
                zs[t] = z
                for half in range(gwg // PW):
                    if last and t == TLOC - 1 and half == gwg // PW - 1:
                        # final half: mm1 in full, then 512-col silu /
                        # mm2 / evac / store quarters so the tail chain
                        # after the very last silu is short
                        p1 = ppool.tile([128, PW], fp32, tag="ps")
                        emit_mm1_half(t, c0, p1, half * PW)
                        # earlier halves of the last group: their mm2
                        # inputs complete during the mm1 above; they
                        # hide under the final silus from here
                        for zoff in range(0, gwg - PW, PW):
                            emit_mm2_half(c0, zs, zoff, fine=True)
                        o = opool.tile([128, PW], fp32, tag="o")
                        p2 = ppool.tile([128, PW], fp32, tag="ps")
                        for q in range(2):
                            zoff = half * PW + q * MMN
                            emit_silu(z, p1, t, zoff, width=MMN, poff=q * MMN)
                            emit_mm2_q(zs, p2, zoff, q * MMN)
                            nc.vector.tensor_scalar_add(
                                o[:, q * MMN : (q + 1) * MMN],
                                p2[:, q * MMN : (q + 1) * MMN],
                                0.0,
                            )
                            nc.sync.dma_start(
                                out.ap()[:, c0 + zoff : c0 + zoff + MMN],
                                o[0:97:32, q * MMN : (q + 1) * MMN],
                            )
                        continue
                    p1 = ppool.tile([128, PW], fp32, tag="ps")
                    if gi == 0 and t <= 2 and half == 0:
                        # earliest tiles: k-inner quarters + 512-col
                        # silus -- the scalar stream has mm1-gated holes
                        # until it saturates (~g1), so firing each silu
                        # half an mm1 earlier recovers hole time and the
                        # extra ACTIVATE overhead lands inside the holes
                        for hh in range(PW // MMN):
                            hc = hh * MMN
                            for k in range(KCH):
                                nc.tensor.matmul(
                                    p1[:, hc : hc + MMN],
                                    w1sb[:, (t * KCH + k) * H : (t * KCH + k + 1) * H],
                                    xt8[t][:, k, c0 + hc : c0 + hc + MMN],
                                    start=(k == 0),
                                    stop=(k == KCH - 1),
                                )
                            emit_silu(z, p1, t, hc, width=MMN, poff=hc)
                        continue
                    emit_mm1_half(t, c0, p1, half * PW)
                    emit_silu(z, p1, t, half * PW)

                if t == 0 and pend is not None:
                    # previous group's second matmul, emitted after this
                    # group's first head so its ~1.8us of X-free PE work
                    # gives the DMA stream breathing room
                    pc0, pwidth, pzs = pend
                    for zoff in range(0, pwidth, PW):
                        emit_mm2_half(pc0, pzs, zoff)
                    pend = None
            if not last:
                pend = (c0, gwg, zs)


    nc.compile()
    return nc


def make_in_maps(states_batch, W1, b1, W2, b2):
    import ml_dtypes

    states_batch = np.asarray(states_batch)
    W1, b1, W2, b2 = (np.asarray(a) for a in (W1, b1, W2, b2))
    b_total = states_batch.shape[1]
    in_maps = []
    for c in range(NCORES):
        sl = slice(c * TLOC, (c + 1) * TLOC)
        xt = states_batch[sl].transpose(0, 2, 1)  # [TLOC, S, B]
        m = {}
        # [t, k, p, cols] -> [t, p, k, cols] so one 3D DMA lands both chunks
        m["x8"] = np.ascontiguousarray(
            xt.reshape(TLOC, KCH, 128, b_total).transpose(0, 2, 1, 3)
        ).astype(ml_dtypes.float8_e3m4)
        m["w1"] = (
            W1[sl]
            .reshape(TLOC, KCH, 128, H)
            .transpose(2, 0, 1, 3)
            .reshape(128, TLOC * KCH * H)
            .astype(np.float16)
        )
        m["b1"] = np.ascontiguousarray(b1[sl].T).astype(np.float32)
        m["w2"] = np.repeat(
            np.ascontiguousarray(W2[sl][:, :, 0].T).astype(np.float16), 32, axis=1
        )
        in_maps.append(m)
    return in_maps


def run(inputs: dict, trace: bool = False):
    from concourse import bass_utils

    nc = build_nc()
    in_maps = make_in_maps(**inputs)
    kw = {"tmpdir": "/tmp/ntff"} if trace else {}
    res = bass_utils.run_bass_kernel_spmd(
        nc, in_maps, core_ids=list(range(NCORES)), trace=trace, **kw
    )
    out = np.concatenate([r["out"] for r in res.results], axis=0)
    # b2 bias is a [T,1] broadcast; applied here rather than on-device
    out = (out + np.asarray(inputs["b2"]).astype(np.float32)).astype(np.float32)
    return out, res


def kernel(**inputs) -> np.ndarray:
    out, _ = run(inputs)
    return out
